# revision 1
# baseline (speedup 1.0000x reference)
"""Bahdanau-attention decoder cell (GRU-style) on 8 Trainium2 NeuronCores.

Sharding: data-parallel over batch. Each of the 8 cores processes 8 of the
64 examples; all weight matrices are replicated. No collectives needed.

Per-core pipeline (per example b), fp8 mode:
  1. DMA encoder_hiddens[b] (1024x2048 f32) -> SBUF as fp8e4m3, k-tiled
     [128p, 8k, 2048l]  (SWDGE cast during DMA).
  2. enc_scores tile (a,l): psum[128a,512l] = sum_ks DoubleRow fp8 matmuls
     with lhsT = 64*Ua.T tiles (x64 scaling keeps Ua out of fp8 subnormals).
  3. v = tanh(psum/64 + decT[:,m,b])  (ACT, per-partition bias) -> bf16
  4. energies: psum[1,512] += va_col[m].T @ v[m]  (bf16 M=1 matvec)
  5. w = exp(e) (ACT, accum_out gives S); w8 = w * (256/S) -> fp8  (DVE)
  6. w8 columns via PE (lhsT=w8_seg[1,128], rhs=one8[1,1])
  7. h^T per 256-l super-tile via PE transpose (fp8); context psum[1,1024]
     accumulates DoubleRow fp8 matvecs; c row = psum/256 -> bf16
Then batched over the core's 8 examples: GRU gates r,z,s_prop as
[hout,b]-major bf16 matmuls (weights pre-transposed+packed on host),
final combine in f32, PE-transpose back to [b,hout], DMA out.
"""

import numpy as np
import ml_dtypes

import concourse.bass as bass
import concourse.tile as tile
from concourse import bacc
from concourse import mybir
from concourse.bass_utils import run_bass_kernel_spmd
from concourse.masks import make_identity

F32 = mybir.dt.float32
BF16 = mybir.dt.bfloat16
FP8 = mybir.dt.float8e4
U16 = mybir.dt.uint16
AF = mybir.ActivationFunctionType
DR = mybir.MatmulPerfMode.DoubleRow

N_CORES = 8
B, IN, H, A, L = 64, 512, 512, 512, 2048
H2 = 2 * H
BL = B // N_CORES  # examples per core
KA = H2 // 128     # k-tiles over the 2H contraction dim

USE_FP8 = True
UA_SCALE = 64.0    # Ua pre-scale so fp8 values stay out of subnormals
W_SCALE = 256.0    # alpha pre-scale before fp8 cast


def build_decoder_cell(n_ex: int = BL, use_fp8: bool = USE_FP8):
    nc = bacc.Bacc(None, target_bir_lowering=False, debug=True)
    hdt = FP8 if use_fp8 else BF16

    x16 = nc.declare_dram_parameter("x16", [n_ex, IN], BF16, isOutput=False)
    sp16 = nc.declare_dram_parameter("sp16", [n_ex, H], BF16, isOutput=False)
    sp32 = nc.declare_dram_parameter("sp32", [n_ex, H], F32, isOutput=False)
    enc = nc.declare_dram_parameter("enc", [n_ex, H2, L], F32, isOutput=False)
    uaT = nc.declare_dram_parameter("uaT", [128, KA * A], hdt, isOutput=False)
    waT = nc.declare_dram_parameter("waT", [128, 4 * A], BF16, isOutput=False)
    wrT = nc.declare_dram_parameter("wrT", [128, 4 * H], BF16, isOutput=False)
    wzT = nc.declare_dram_parameter("wzT", [128, 4 * H], BF16, isOutput=False)
    wsT = nc.declare_dram_parameter("wsT", [128, 4 * H], BF16, isOutput=False)
    urT = nc.declare_dram_parameter("urT", [128, 4 * H], BF16, isOutput=False)
    uzT = nc.declare_dram_parameter("uzT", [128, 4 * H], BF16, isOutput=False)
    usT = nc.declare_dram_parameter("usT", [128, 4 * H], BF16, isOutput=False)
    crT = nc.declare_dram_parameter("crT", [128, KA * H], BF16, isOutput=False)
    czT = nc.declare_dram_parameter("czT", [128, KA * H], BF16, isOutput=False)
    csT = nc.declare_dram_parameter("csT", [128, KA * H], BF16, isOutput=False)
    va_c = nc.declare_dram_parameter("va_c", [128, 32] if use_fp8 else [128, 4],
                                     FP8 if use_fp8 else BF16, isOutput=False)
    y = nc.declare_dram_parameter("y", [n_ex, H], F32, isOutput=True)

    enc_t = enc[:].rearrange("e (k p) l -> e p k l", p=128)

    with tile.TileContext(nc) as tc:
        with (
            tc.tile_pool(name="singles", bufs=1) as singles,
            tc.tile_pool(name="hpool", bufs=6) as hpool,
            tc.tile_pool(name="vpool", bufs=12) as vpool,
            tc.tile_pool(name="htpool", bufs=10) as htpool,
            tc.tile_pool(name="smpool", bufs=3) as smpool,
            tc.tile_pool(name="ps_mm", bufs=2, space="PSUM") as ps_mm,
            tc.tile_pool(name="ps_e", bufs=1, space="PSUM") as ps_e,
            tc.tile_pool(name="ps_t", bufs=3, space="PSUM") as ps_t,
            tc.tile_pool(name="ps_c", bufs=1, space="PSUM") as ps_c,
        ):
            # ---- one-time setup ----
            idh = singles.tile([128, 128], hdt)
            make_identity(nc, idh)
            id128f = singles.tile([128, 128], F32)
            make_identity(nc, id128f)
            idb = singles.tile([n_ex, n_ex], BF16)
            make_identity(nc, idb)
            idbf = singles.tile([n_ex, n_ex], F32)
            make_identity(nc, idbf)
            one1 = singles.tile([1, 1], hdt)
            nc.vector.memset(one1, 1.0)
            oneb = singles.tile([1, 1], BF16)
            nc.vector.memset(oneb, 1.0)
            onef = singles.tile([1, 1], F32)
            nc.vector.memset(onef, 1.0)
            ones_row = singles.tile([1, 128], F32)
            nc.vector.memset(ones_row, 1.0)
            ones_col = singles.tile([128, 1], F32)
            nc.vector.memset(ones_col, 1.0)

            uaT_sb = singles.tile([128, KA, A], hdt)
            nc.sync.dma_start(out=uaT_sb, in_=uaT[:].rearrange("p (k a) -> p k a", k=KA))
            waT_sb = singles.tile([128, 4 * A], BF16)
            nc.sync.dma_start(out=waT_sb, in_=waT[:])
            gate_w = {}
            for nm, dram in [("wrT", wrT), ("wzT", wzT), ("wsT", wsT),
                             ("urT", urT), ("uzT", uzT), ("usT", usT)]:
                t = singles.tile([128, 4 * H], BF16, name=nm + "_sb")
                nc.sync.dma_start(out=t, in_=dram[:])
                gate_w[nm] = t
            for nm, dram in [("crT", crT), ("czT", czT), ("csT", csT)]:
                t = singles.tile([128, KA * H], BF16, name=nm + "_sb")
                nc.sync.dma_start(out=t, in_=dram[:])
                gate_w[nm] = t
            if use_fp8:
                va_sb = singles.tile([128, 2, 16], FP8)
                nc.sync.dma_start(out=va_sb, in_=va_c[:].rearrange(
                    "p (two j) -> p two j", two=2))
            else:
                va_sb = singles.tile([128, 4], BF16)
                nc.sync.dma_start(out=va_sb, in_=va_c[:])

            x16_sb = singles.tile([n_ex, IN], BF16)
            nc.sync.dma_start(out=x16_sb, in_=x16[:])
            sp16_sb = singles.tile([n_ex, H], BF16)
            nc.sync.dma_start(out=sp16_sb, in_=sp16[:])
            sp32_sb = singles.tile([n_ex, H], F32)
            nc.sync.dma_start(out=sp32_sb, in_=sp32[:])

            # transpose x / sprev to [feat-part, k, b]
            xT_sb = singles.tile([128, 4, n_ex], BF16)
            spT16_sb = singles.tile([128, 4, n_ex], BF16)
            spT32_sb = singles.tile([128, 4, n_ex], F32)
            for j in range(4):
                pst = ps_t.tile([128, 512], BF16, tag="ps_t", name="pst_x")
                nc.tensor.transpose(pst[:, :n_ex], x16_sb[:, j * 128:(j + 1) * 128], idb)
                nc.scalar.copy(xT_sb[:, j, :], pst[:, :n_ex])
                pst2 = ps_t.tile([128, 512], BF16, tag="ps_t", name="pst_s")
                nc.tensor.transpose(pst2[:, :n_ex], sp16_sb[:, j * 128:(j + 1) * 128], idb)
                nc.scalar.copy(spT16_sb[:, j, :], pst2[:, :n_ex])
                pst3 = ps_t.tile([128, 512], F32, tag="ps_t", name="pst_s32")
                nc.tensor.transpose(pst3[:, :n_ex], sp32_sb[:, j * 128:(j + 1) * 128], idbf)
                nc.scalar.copy(spT32_sb[:, j, :], pst3[:, :n_ex])

            cT_sb = singles.tile([128, KA, n_ex], BF16)

            # decT[a, b] = (sprev @ Wa.T).T
            decT_sb = singles.tile([128, 4, n_ex], F32)
            for m in range(4):
                ps = ps_mm.tile([128, 512], F32, tag="ps_mm", name="ps_dec")
                for k in range(4):
                    nc.tensor.matmul(
                        ps[:, :n_ex],
                        lhsT=waT_sb[:, k * A + m * 128:k * A + (m + 1) * 128],
                        rhs=spT16_sb[:, k, :],
                        start=(k == 0), stop=(k == 3),
                    )
                nc.scalar.copy(decT_sb[:, m, :], ps[:, :n_ex])

            # ---- per-example attention ----
            for b in range(n_ex):
                h_halves = []
                for hf in range(2):
                    ht = hpool.tile([128, KA, L // 2], hdt, tag="h",
                                    name=f"h_{b}_{hf}")
                    nc.gpsimd.dma_start(
                        out=ht, in_=enc_t[b][:, :, hf * (L // 2):(hf + 1) * (L // 2)])
                    h_halves.append(ht)

                e_sb = smpool.tile([1, L], F32, tag="e", name=f"e_{b}")
                for lc in range(4):
                    if use_fp8:
                        v_lc = vpool.tile([128, 4, 512], FP8, tag="v",
                                          name=f"v{b}_{lc}")
                    vs = []
                    for m in range(4):
                        ps = ps_mm.tile([128, 512], F32, tag="ps_mm", name=f"ps_s{b}_{lc}_{m}")
                        if use_fp8:
                            for ks in range(KA // 2):
                                nc.tensor.matmul(
                                    ps,
                                    lhsT=uaT_sb[:, 2 * ks:2 * ks + 2, m * 128:(m + 1) * 128],
                                    rhs=h_halves[lc // 2][:, 2 * ks:2 * ks + 2,
                                        (lc % 2) * 512:(lc % 2 + 1) * 512],
                                    start=(ks == 0), stop=(ks == KA // 2 - 1),
                                    perf_mode=DR,
                                )
                        else:
                            for k in range(KA):
                                nc.tensor.matmul(
                                    ps,
                                    lhsT=uaT_sb[:, k, m * 128:(m + 1) * 128],
                                    rhs=h_halves[lc // 2][:, k,
                                        (lc % 2) * 512:(lc % 2 + 1) * 512],
                                    start=(k == 0), stop=(k == KA - 1),
                                )
                        if use_fp8:
                            v = v_lc[:, m, :]
                        else:
                            v = vpool.tile([128, 512], BF16, tag="v",
                                           name=f"v{b}_{lc}_{m}")
                        nc.scalar.activation(v, ps, AF.Tanh,
                                             bias=decT_sb[:, m, b:b + 1],
                                             scale=(1.0 / UA_SCALE) if use_fp8 else 1.0)
                        vs.append(v)
                    eps = ps_e.tile([1, 512], F32, tag="ps_e", name=f"eps{b}_{lc}")
                    if use_fp8:
                        for q in range(2):
                            nc.tensor.matmul(eps, lhsT=va_sb[:, :, q:q + 1],
                                             rhs=v_lc[:, 2 * q:2 * q + 2, :],
                                             start=(q == 0), stop=(q == 1),
                                             perf_mode=DR)
                        # e was accumulated at VA_SCALE; undo on copy-out
                        nc.scalar.mul(e_sb[:, lc * 512:(lc + 1) * 512], eps,
                                      1.0 / UA_SCALE)
                    else:
                        for m in range(4):
                            nc.tensor.matmul(eps, lhsT=va_sb[:, m:m + 1], rhs=vs[m],
                                             start=(m == 0), stop=(m == 3))
                        nc.scalar.copy(e_sb[:, lc * 512:(lc + 1) * 512], eps)

                if use_fp8:
                    # h^T for all 16 l-tiles up front (PE + copies overlap the
                    # softmax chain below); held in htpool until the ctx MMs
                    hts = []
                    for ltp in range(8):
                        hT_sb = htpool.tile([128, 2, H2], FP8, tag="ht", name=f"hT_{b}_{ltp}")
                        for q in range(4):
                            ko, hh = q // 2, q % 2
                            lt = 2 * ltp + ko
                            htp = ps_t.tile([128, 512], F32, tag="ps_t",
                                            name=f"htp_{b}_{ltp}_{q}")
                            for kj in range(4):
                                kk = hh * 4 + kj
                                nc.tensor.matmul(
                                    htp[:, kj * 128:(kj + 1) * 128],
                                    lhsT=h_halves[lt // 8][:, kk,
                                                   (lt % 8) * 128:(lt % 8 + 1) * 128],
                                    rhs=idh, start=True, stop=True,
                                )
                            dst = hT_sb[:, ko, hh * 512:(hh + 1) * 512]
                            if (ltp * 4 + q) % 16 in (0, 5, 10):
                                nc.scalar.copy(dst, htp)
                            else:
                                nc.vector.tensor_copy(dst, htp)
                        hts.append(hT_sb)

                    et_ps = ps_t.tile([128, 512], F32, tag="ps_t", name=f"etps_{b}")
                    for t in range(16):
                        col = (t % 2) * 16 + t // 2
                        nc.tensor.matmul(et_ps[:, col:col + 1],
                                         lhsT=e_sb[:, t * 128:(t + 1) * 128],
                                         rhs=onef, start=True, stop=True)
                    et_v = et_ps[:, :32].rearrange("p (two j) -> p two j", two=2)
                    w_sb = smpool.tile([128, 2, 8], F32, tag="w", name=f"w_{b}")
                    psum_sb = smpool.tile([128, 1], F32, tag="S", name=f"S_{b}")
                    nc.scalar.activation(w_sb, et_v[:, :, :8], AF.Exp,
                                         accum_out=psum_sb)
                    # S = sum over partitions; invs_col = (256/S) broadcast
                    stot_ps = ps_e.tile([1, 512], F32, tag="ps_e", name=f"stot_{b}")
                    nc.tensor.matmul(stot_ps[:, :1], lhsT=psum_sb, rhs=ones_col,
                                     start=True, stop=True)
                    invs_sb = smpool.tile([1, 1], F32, tag="invS", name=f"invS_{b}")
                    nc.vector.reciprocal(invs_sb, stot_ps[:, :1])
                    invs2_sb = smpool.tile([1, 1], F32, tag="invS2", name=f"invS2_{b}")
                    nc.vector.tensor_scalar_mul(invs2_sb, in0=invs_sb, scalar1=W_SCALE)
                    ibc_ps = ps_t.tile([128, 512], F32, tag="ps_t", name=f"ibc_{b}")
                    nc.tensor.matmul(ibc_ps[:, :1], lhsT=ones_row, rhs=invs2_sb,
                                     start=True, stop=True)
                    invc_sb = smpool.tile([128, 1], F32, tag="invc", name=f"invc_{b}")
                    nc.scalar.copy(invc_sb, ibc_ps[:, :1])
                    wT_sb = smpool.tile([128, 2, 16], FP8, tag="wT", name=f"wT_{b}")
                    nc.vector.tensor_scalar_mul(wT_sb[:, :, :8], in0=w_sb,
                                                scalar1=invc_sb)

                    # context: cT[1, H2] += DoubleRow fp8 over 256-l super-tiles
                    # (h^T tiles already produced above; only w8 gates these)
                    ct_ps = ps_c.tile([1, H2], F32, tag="ps_c", name=f"ctps_{b}")
                    for ltp in range(8):
                        for half in range(2):
                            nc.tensor.matmul(
                                ct_ps[:, half * 512:(half + 1) * 512],
                                lhsT=wT_sb[:, :, ltp:ltp + 1],
                                rhs=hts[ltp][:, :, half * 512:(half + 1) * 512],
                                start=(ltp == 0), stop=(ltp == 7),
                                perf_mode=DR,
                            )
                    c_row_sb = smpool.tile([1, H2], BF16, tag="crow", name=f"crow_{b}")
                    nc.scalar.activation(c_row_sb, ct_ps, AF.Copy, scale=1.0 / W_SCALE)
                else:
                    wt_ps = ps_t.tile([128, 512], F32, tag="ps_t", name=f"wtps_{b}")
                    for t in range(16):
                        nc.tensor.matmul(wt_ps[:, t:t + 1],
                                         lhsT=w_sb[:, t * 128:(t + 1) * 128],
                                         rhs=one1, start=True, stop=True)
                    wT_sb = smpool.tile([128, 16], BF16, tag="wT", name=f"wT_{b}")
                    nc.vector.tensor_copy(wT_sb, wt_ps[:, :16])

                    ct_ps = ps_c.tile([1, H2], F32, tag="ps_c", name=f"ctps_{b}")
                    for lt in range(16):
                        htp = ps_t.tile([128, H2], BF16, tag="ps_t", name=f"htp_{b}_{lt}")
                        for kk in range(KA):
                            nc.tensor.transpose(
                                htp[:, kk * 128:(kk + 1) * 128],
                                h_halves[lt // 8][:, kk,
                                         (lt % 8) * 128:(lt % 8 + 1) * 128],
                                idh,
                            )
                        hT_sb = htpool.tile([128, H2], BF16, tag="ht", name=f"hT_{b}_{lt}")
                        if lt % 2 == 0:
                            nc.scalar.copy(hT_sb, htp)
                        else:
                            nc.vector.tensor_copy(hT_sb, htp)
                        for half in range(2):
                            nc.tensor.matmul(
                                ct_ps[:, half * 512:(half + 1) * 512],
                                lhsT=wT_sb[:, lt:lt + 1],
                                rhs=hT_sb[:, half * 512:(half + 1) * 512],
                                start=(lt == 0), stop=(lt == 15),
                            )
                    c_row_sb = smpool.tile([1, H2], BF16, tag="crow", name=f"crow_{b}")
                    nc.scalar.activation(c_row_sb, ct_ps, AF.Copy, scale=invs_sb)

                # scatter c into column-major cT_sb[:, j, b] via K=1 matmuls
                ctt_ps = ps_t.tile([128, 512], F32, tag="ps_t", name=f"cttps_{b}")
                for j in range(KA):
                    nc.tensor.matmul(ctt_ps[:, j:j + 1],
                                     lhsT=c_row_sb[:, j * 128:(j + 1) * 128],
                                     rhs=oneb, start=True, stop=True)
                nc.vector.tensor_copy(cT_sb[:, :, b:b + 1], ctt_ps[:, :KA])

            # ---- batched GRU over the core's examples ----
            def gate_psum(wname, uname, cname, u_rhs, name):
                """Yields psum[hout-tile m, b] = W.T@xT + U.T@u_rhs + C.T@cT."""
                for m in range(4):
                    ps = ps_mm.tile([128, 512], F32, tag="ps_mm", name=f"{name}_{m}")
                    wt, ut, ct = gate_w[wname], gate_w[uname], gate_w[cname]
                    for k in range(4):
                        nc.tensor.matmul(
                            ps[:, :n_ex], lhsT=wt[:, k * H + m * 128:k * H + (m + 1) * 128],
                            rhs=xT_sb[:, k, :], start=(k == 0), stop=False)
                    for k in range(4):
                        nc.tensor.matmul(
                            ps[:, :n_ex], lhsT=ut[:, k * H + m * 128:k * H + (m + 1) * 128],
                            rhs=u_rhs[:, k, :], start=False, stop=False)
                    for k in range(KA):
                        nc.tensor.matmul(
                            ps[:, :n_ex], lhsT=ct[:, k * H + m * 128:k * H + (m + 1) * 128],
                            rhs=cT_sb[:, k, :], start=False, stop=(k == KA - 1))
                    yield m, ps

            r_sb = singles.tile([128, 4, n_ex], F32)
            rs16_sb = singles.tile([128, 4, n_ex], BF16)
            for m, ps in gate_psum("wrT", "urT", "crT", spT16_sb, "ps_r"):
                nc.scalar.activation(r_sb[:, m, :], ps[:, :n_ex], AF.Sigmoid)
                nc.vector.tensor_mul(rs16_sb[:, m, :], r_sb[:, m, :], spT32_sb[:, m, :])

            z_sb = singles.tile([128, 4, n_ex], F32)
            for m, ps in gate_psum("wzT", "uzT", "czT", spT16_sb, "ps_z"):
                nc.scalar.activation(z_sb[:, m, :], ps[:, :n_ex], AF.Sigmoid)

            outT_sb = singles.tile([128, 4, n_ex], F32)
            d_sb = singles.tile([128, 4, n_ex], F32)
            for m, ps in gate_psum("wsT", "usT", "csT", rs16_sb, "ps_p"):
                sp_prop = singles.tile([128, n_ex], F32, name=f"spp_{m}")
                nc.scalar.activation(sp_prop, ps[:, :n_ex], AF.Tanh)
                # out = sprev + z * (s_prop - sprev)
                nc.vector.tensor_sub(d_sb[:, m, :], sp_prop, spT32_sb[:, m, :])
                nc.vector.tensor_mul(d_sb[:, m, :], d_sb[:, m, :], z_sb[:, m, :])
                nc.vector.tensor_add(outT_sb[:, m, :], d_sb[:, m, :], spT32_sb[:, m, :])

            o_ps = ps_t.tile([128, 512], F32, tag="ps_t", name="o_ps")
            for m in range(4):
                nc.tensor.transpose(o_ps[:n_ex, m * 128:(m + 1) * 128],
                                    outT_sb[:, m, :], id128f)
            y_sb = singles.tile([n_ex, H], F32)
            nc.scalar.copy(y_sb, o_ps[:n_ex, :])
            nc.sync.dma_start(out=y[:], in_=y_sb)

    nc.compile()
    return nc


def _pack(wT: np.ndarray) -> np.ndarray:
    """[K, M] (K = contraction) -> [128, (K//128)*M] with slice
    [:, k*M + j] == wT[k*128 + p, j]."""
    K, M = wT.shape
    return np.ascontiguousarray(
        wT.reshape(K // 128, 128, M).transpose(1, 0, 2).reshape(128, -1))


def _pack_va(va: np.ndarray) -> np.ndarray:
    if not USE_FP8:
        return np.ascontiguousarray(va.astype(ml_dtypes.bfloat16).reshape(4, 128).T)
    out = np.zeros((128, 2, 16), dtype=ml_dtypes.float8_e4m3fn)
    for q in range(2):
        for ko in range(2):
            out[:, ko, q] = (va[(2 * q + ko) * 128:(2 * q + ko + 1) * 128]
                             * UA_SCALE).astype(ml_dtypes.float8_e4m3fn)
    return out.reshape(128, 32)


_BUILT = {}


def _get_nc(n_ex: int):
    if n_ex not in _BUILT:
        _BUILT[n_ex] = build_decoder_cell(n_ex)
    return _BUILT[n_ex]


LAST_RESULTS = None


def kernel(x, sprev, encoder_hiddens, Ws, Wz, Wr, Us, Uz, Ur,
           Cs, Cz, Cr, bs, bz, br, va, Wa, Ua, _trace=False) -> np.ndarray:
    global LAST_RESULTS
    bf = ml_dtypes.bfloat16
    f8 = ml_dtypes.float8_e4m3fn
    nc = _get_nc(BL)

    if USE_FP8:
        ua_packed = _pack((Ua.T * UA_SCALE).astype(f8))
    else:
        ua_packed = _pack(Ua.T.astype(bf))
    wmap = {
        "uaT": ua_packed,
        "waT": _pack(Wa.T.astype(bf)),
        "wrT": _pack(Wr.T.astype(bf)),
        "wzT": _pack(Wz.T.astype(bf)),
        "wsT": _pack(Ws.T.astype(bf)),
        "urT": _pack(Ur.T.astype(bf)),
        "uzT": _pack(Uz.T.astype(bf)),
        "usT": _pack(Us.T.astype(bf)),
        "crT": _pack(Cr.T.astype(bf)),
        "czT": _pack(Cz.T.astype(bf)),
        "csT": _pack(Cs.T.astype(bf)),
        "va_c": _pack_va(va),
    }
    in_maps = []
    for i in range(N_CORES):
        sl = slice(i * BL, (i + 1) * BL)
        in_maps.append({
            "x16": x[sl].astype(bf),
            "sp16": sprev[sl].astype(bf),
            "sp32": np.ascontiguousarray(sprev[sl]),
            "enc": np.ascontiguousarray(encoder_hiddens[sl]),
            **wmap,
        })
    res = run_bass_kernel_spmd(nc, in_maps, core_ids=list(range(N_CORES)),
                               trace=_trace)
    LAST_RESULTS = res
    return np.concatenate([res.results[i]["y"] for i in range(N_CORES)], axis=0)

